# revision 16
# baseline (speedup 1.0000x reference)
"""BackgroundNoiseLayer kernel for 8 trn2 NeuronCores.

Math: out[0, t, n] = sum_k W[n, k] * rest[t, k], where W [60000, 100] is
scatter-added from COO (v1 block rows 0..49999, lm block rows 50000..59999)
and the output feature axis is the concat of the two blocks.

Strategy (per sharding hint): densify the tiny sparse matrix host-side
(240k nnz -> dense W, ~0.002% of the matmul FLOPs), shard the post-synaptic
feature axis across the 8 cores (7500 features each), and run a dense
[1000,100] @ [100,7500] matmul per core. rest is tiny and replicated. Each
core writes its own contiguous output slice; concat on host is the no-op
gather.

Precision budget: the gate is rel_err < 2e-2. A single bf16 weight plane
(8 mantissa bits) + bf16 output quantization lands at ~2e-3 — an order of
magnitude inside tolerance — so the kernel runs ONE bf16 matmul per tile
(not a 3-plane fp32-exact split) and streams the output as bf16, halving
the dominant HBM write (30.7 MB -> 15.4 MB f32->bf16 per core). rest
(Poisson spike counts, small ints) is bf16-exact; a host-side sparse
correction covers any value that is not.

Device-side pipeline (from profiling; out-wire ~42 us is the roofline):
- Shapes zero-padded to DMA-friendly sizes (K 100->112, rows 1000->1024,
  features/core 7500->7680=15*512): DMAs whose partition count is not a
  multiple of 16 measured ~2.7x slower, and 512-f32 matmul chunks land
  exactly on PSUM bank boundaries. The last row block holds only pad rows
  past 1007, so its out-DMA writes 112 partitions instead of 128.
- All out-DMAs ride the single sync (HWDGE) ring: splitting them across
  two rings measured ~25% wire inflation (SDMA engines round-robin
  between queues at packet granularity), and full-row-block DMAs are
  fully contiguous 1.92 MB HBM writes.
- stage bufs=3 so copies for block N never wait on block N-2's out-DMA
  write receipt (~2 us completion latency on top of wire time) — with
  bufs=2 that wait showed up as 2-4 us sync-sequencer stalls per block.
- PSUM->SBUF drains alternate vector/scalar (gpsimd cannot read PSUM);
  the PSUM read port (1 elem/cycle/partition) makes the combined drain
  ~37 us — just under the out-wire, so it pipelines.
"""

import os

import numpy as np

B, T = 1, 1000
NBKG = 100
NV1, NLM = 50000, 10000
NPOST = NV1 + NLM          # 60000
NCORES = 8
SHARD = NPOST // NCORES    # 7500 real features per core

KP = 112                   # padded contraction dim (zeros in rows 100..111)
ROWS = 1024                # padded time rows (zeros in 1000..1023)
OUTP = 7680                # padded features per core = 15 * 512
TBLK = 128                 # rows per block = full partition set
NT = ROWS // TBLK          # 8
LASTP = 112                # partitions written by the last block (rows ..1007)
MMN = 512                  # matmul free dim = exactly one fp32 PSUM bank
NCH = OUTP // MMN          # 15 chunks per row block
NMM = 3                    # chunks per ramp out-piece
WSPLIT = (512, 1024, 1536, 1536, 1536, 1536)   # w in-DMA chunk widths
RESTW = (TBLK, ROWS - TBLK)                    # rest in-DMA chunk widths
# matmul chunk c -> (w tile index, col offset within tile)
WMAP = []
for _wi, _w in enumerate(WSPLIT):
    for _o in range(0, _w, MMN):
        WMAP.append((_wi, _o))

_compiled = None


def _build_module():
    import concourse.bacc as bacc
    import concourse.mybir as mybir
    import concourse.tile as tile

    f32 = mybir.dt.float32
    bf16 = mybir.dt.bfloat16
    nc = bacc.Bacc("TRN2", target_bir_lowering=False, debug=False)
    # inputs split into per-ring chunks: the first matmul needs only w cols
    # 0..511 and rest cols 0..127, so those are tiny lead chunks whose DMA
    # completion (wire + ~2 us write receipt) gates the pipeline start
    rest_d = [nc.dram_tensor(f"rest{i}", [KP, w], bf16, kind="ExternalInput")
              for i, w in enumerate(RESTW)]
    w_d = [nc.dram_tensor(f"w{i}", [KP, w], bf16, kind="ExternalInput")
           for i, w in enumerate(WSPLIT)]
    out = nc.dram_tensor("out", [ROWS, SHARD], bf16, kind="ExternalOutput")

    with tile.TileContext(nc) as tc:
        with (
            tc.tile_pool(name="inp", bufs=1) as inp,
            tc.tile_pool(name="stage", bufs=3) as stagep,
            tc.tile_pool(name="psum", bufs=8, space="PSUM") as psump,
        ):
            # balanced across the three rings so every chunk's receipt
            # (~2 us after its last byte) fires before its first consuming
            # matmul: each ring leads with the earliest-needed chunk.
            #   sync:   w0 (512, gates matmul c=0), w2, w4
            #   scalar: rest0 (block-0 cols, gates first LDWEIGHTS), w3
            #   gpsimd: w1 (gates matmul c=1), rest1, w5
            rest_sb = [inp.tile([KP, w], bf16, tag=f"rest{i}",
                                name=f"rest{i}")
                       for i, w in enumerate(RESTW)]
            w_sb = [inp.tile([KP, w], bf16, tag=f"w{i}", name=f"w{i}")
                    for i, w in enumerate(WSPLIT)]
            nc.sync.dma_start(w_sb[0][:], w_d[0][:])
            nc.scalar.dma_start(rest_sb[0][:], rest_d[0][:])
            nc.gpsimd.dma_start(w_sb[1][:], w_d[1][:])
            nc.sync.dma_start(w_sb[2][:], w_d[2][:])
            nc.scalar.dma_start(w_sb[3][:], w_d[3][:])
            nc.sync.dma_start(w_sb[4][:], w_d[4][:])
            nc.gpsimd.dma_start(rest_sb[1][:], rest_d[1][:])
            nc.gpsimd.dma_start(w_sb[5][:], w_d[5][:])

            copy_engines = [nc.vector.tensor_copy, nc.scalar.copy]
            # pieced blocks stream column pieces from their own small
            # tiles (a piece DMA waits on its whole tile, so shared tiles
            # would gate on all 15 copies). Ramp blocks 0-2 (fine 3-chunk
            # pieces) ride the otherwise-idle gpsimd SWDGE ring so their
            # 15 descriptor pushes don't head-of-line block the sync ring;
            # block 3 (5-chunk pieces) bridges into the steady phase and
            # block 7's pieces shorten the non-overlappable tail DMA.
            # Steady blocks 4-6 use whole-row contiguous 1.92 MB DMAs
            # from a triple-buffered full-width stage.
            for tb in range(NT):
                outp = LASTP if tb == NT - 1 else TBLK
                if tb < 3:
                    pieces, ring = [3, 3, 3, 3, 3], nc.gpsimd
                elif tb == 3:
                    pieces, ring = [5, 5, 5], nc.sync
                elif tb == NT - 1:
                    # short last piece: its wire time is the only
                    # non-overlappable part of the whole out stream
                    pieces, ring = [6, 6, 3], nc.sync
                else:
                    pieces, ring = None, nc.sync
                    stage = stagep.tile([TBLK, OUTP], bf16, tag="stage",
                                        name=f"stage{tb}", bufs=3)
                    base = 0
                if tb == 0:
                    lhsT = rest_sb[0][:]
                else:
                    lhsT = rest_sb[1][:, (tb - 1) * TBLK:tb * TBLK]
                bounds = []
                if pieces is not None:
                    acc = 0
                    for p in pieces:
                        bounds.append((acc, acc + p))
                        acc += p
                for c in range(NCH):
                    if pieces is not None and any(c == b[0] for b in bounds):
                        pi = [i for i, b in enumerate(bounds)
                              if b[0] == c][0]
                        base = c * MMN
                        pw = pieces[pi] * MMN
                        stage = stagep.tile([TBLK, pw], bf16,
                                            tag=f"p{tb}_{pi}",
                                            name=f"p{tb}_{pi}", bufs=1)
                    ps = psump.tile([TBLK, MMN], f32, tag="ps")
                    wi, off = WMAP[c]
                    nc.tensor.matmul(
                        ps[:],
                        lhsT,
                        w_sb[wi][:, off:off + MMN],
                        start=True,
                        stop=True,
                    )
                    fd = min(MMN, SHARD - c * MMN)
                    soff = c * MMN - base
                    copy_engines[(tb * NCH + c) % 2](
                        stage[:, soff:soff + fd], ps[:, :fd]
                    )
                    if pieces is not None and any(c == b[1] - 1
                                                  for b in bounds):
                        lo = base
                        hi = min(c * MMN + MMN, SHARD)
                        ring.dma_start(
                            out[tb * TBLK:tb * TBLK + outp, lo:hi],
                            stage[:outp, :hi - lo],
                        )
                if pieces is None:
                    nc.sync.dma_start(
                        out[tb * TBLK:tb * TBLK + outp, :],
                        stage[:outp, :SHARD],
                    )

    nc.compile()
    return nc


def _densify(v1_weights, v1_rows, v1_cols, lm_weights, lm_rows, lm_cols):
    rows = np.concatenate([
        np.asarray(v1_rows).astype(np.int64),
        np.asarray(lm_rows).astype(np.int64) + NV1,
    ])
    cols = np.concatenate([
        np.asarray(v1_cols).astype(np.int64),
        np.asarray(lm_cols).astype(np.int64),
    ])
    w = np.concatenate([
        np.asarray(v1_weights, dtype=np.float32),
        np.asarray(lm_weights, dtype=np.float32),
    ])
    W = np.bincount(rows * NBKG + cols, weights=w, minlength=NPOST * NBKG)
    return W.astype(np.float32).reshape(NPOST, NBKG)


def kernel(rest, v1_weights, v1_rows, v1_cols, lm_weights, lm_rows, lm_cols):
    import ml_dtypes

    from concourse.bass_utils import run_bass_kernel_spmd

    bf16 = ml_dtypes.bfloat16

    global _compiled
    if _compiled is None:
        _compiled = _build_module()

    W = _densify(v1_weights, v1_rows, v1_cols, lm_weights, lm_rows, lm_cols)
    w_b = W.astype(bf16)

    rest32 = np.asarray(rest, np.float32)
    rest_b = rest32.astype(bf16)

    restT = np.zeros((KP, ROWS), bf16)
    restT[:NBKG, :B * T] = rest_b.T
    rest_chunks = {}
    lo = 0
    for i, w in enumerate(RESTW):
        rest_chunks[f"rest{i}"] = np.ascontiguousarray(restT[:, lo:lo + w])
        lo += w

    in_maps = []
    for c in range(NCORES):
        plane = np.zeros((KP, OUTP), bf16)
        plane[:NBKG, :SHARD] = w_b[c * SHARD:(c + 1) * SHARD].T
        m = dict(rest_chunks)
        lo = 0
        for i, w in enumerate(WSPLIT):
            m[f"w{i}"] = np.ascontiguousarray(plane[:, lo:lo + w])
            lo += w
        in_maps.append(m)

    trace = bool(int(os.environ.get("KERNEL_TRACE", "0")))
    if trace:
        _install_ntff_shim()
    res = run_bass_kernel_spmd(
        _compiled, in_maps, core_ids=list(range(NCORES)), trace=trace
    )
    kernel.last_results = res
    full = np.concatenate(
        [res.results[c]["out"][:B * T, :].astype(np.float32)
         for c in range(NCORES)], axis=1
    )

    # sparse host correction for any rest value that bf16 can't represent
    # exactly (Poisson counts are small ints, so normally there are none)
    rest_err = rest32 - rest_b.astype(np.float32)
    if np.any(rest_err):
        ts, ks = np.nonzero(rest_err)
        for t, k in zip(ts, ks):
            full[t, :] += rest_err[t, k] * W[:, k]

    return full.reshape(B, T, NPOST)


def _install_ntff_shim():
    """The agent image's antenv lacks axon_hooks; register the NTFF profile
    hook by dlopening libaxon_pjrt.so directly (same path trn_boot uses)."""
    import sys
    import types

    if "antenv.axon_hooks" in sys.modules:
        return
    try:
        from trn_agent_boot.trn_boot import _ntff_profile_via_ctypes

        hook = _ntff_profile_via_ctypes("/opt/axon/libaxon_pjrt.so")
    except Exception:
        hook = None
    mod = types.ModuleType("antenv.axon_hooks")
    mod.get_axon_ntff_profile_hook = lambda: hook
    mod.set_axon_ntff_profile_hook = lambda h: None
    sys.modules["antenv.axon_hooks"] = mod


# revision 21
# speedup vs baseline: 1.0573x; 1.0573x over previous
"""BackgroundNoiseLayer kernel for 8 trn2 NeuronCores.

Math: out[0, t, n] = sum_k W[n, k] * rest[t, k], where W [60000, 100] is
scatter-added from COO (v1 block rows 0..49999, lm block rows 50000..59999)
and the output feature axis is the concat of the two blocks.

Strategy (per sharding hint): densify the tiny sparse matrix host-side
(240k nnz -> dense W, ~0.002% of the matmul FLOPs), shard the post-synaptic
feature axis across the 8 cores (7500 features each), and run a dense
[1000,100] @ [100,7500] matmul per core. rest is tiny and replicated. Each
core writes its own contiguous output slice; concat on host is the no-op
gather.

Precision budget: the gate is rel_err < 2e-2. A single bf16 weight plane
(8 mantissa bits) + bf16 output quantization lands at ~2e-3 — an order of
magnitude inside tolerance — so the kernel runs ONE bf16 matmul per tile
(not a 3-plane fp32-exact split) and streams the output as bf16, halving
the dominant HBM write (30.7 MB -> 15.4 MB f32->bf16 per core). rest
(Poisson spike counts, small ints) is bf16-exact; a host-side sparse
correction covers any value that is not.

Device-side pipeline (from profiling; out-wire ~42 us is the roofline):
- Shapes zero-padded to DMA-friendly sizes (K 100->112, rows 1000->1024,
  features/core 7500->7680=15*512): DMAs whose partition count is not a
  multiple of 16 measured ~2.7x slower, and 512-f32 matmul chunks land
  exactly on PSUM bank boundaries. The last row block holds only pad rows
  past 1007, so its out-DMA writes 112 partitions instead of 128.
- All out-DMAs ride the single sync (HWDGE) ring: splitting them across
  two rings measured ~25% wire inflation (SDMA engines round-robin
  between queues at packet granularity), and full-row-block DMAs are
  fully contiguous 1.92 MB HBM writes.
- stage bufs=3 so copies for block N never wait on block N-2's out-DMA
  write receipt (~2 us completion latency on top of wire time) — with
  bufs=2 that wait showed up as 2-4 us sync-sequencer stalls per block.
- PSUM->SBUF drains alternate vector/scalar (gpsimd cannot read PSUM);
  the PSUM read port (1 elem/cycle/partition) makes the combined drain
  ~37 us — just under the out-wire, so it pipelines.
"""

import os

import numpy as np

B, T = 1, 1000
NBKG = 100
NV1, NLM = 50000, 10000
NPOST = NV1 + NLM          # 60000
NCORES = 8
SHARD = NPOST // NCORES    # 7500 real features per core

KP = 112                   # padded contraction dim (zeros in rows 100..111)
ROWS = 1024                # padded time rows (zeros in 1000..1023)
OUTP = 7680                # padded features per core = 15 * 512
TBLK = 128                 # rows per block = full partition set
NT = ROWS // TBLK          # 8
LASTP = 112                # partitions written by the last block (rows ..1007)
MMN = 512                  # matmul free dim = exactly one fp32 PSUM bank
NCH = OUTP // MMN          # 15 chunks per row block
NMM = 3                    # chunks per ramp out-piece
WSPLIT = (1024, 1024, 1536, 1536, 1536, 1024)  # w in-DMA chunk widths
RESTW = (TBLK, ROWS - TBLK)                    # rest in-DMA chunk widths
# matmul chunk c -> (w tile index, col offset within tile)
WMAP = []
for _wi, _w in enumerate(WSPLIT):
    for _o in range(0, _w, MMN):
        WMAP.append((_wi, _o))

_compiled = None


def _build_module():
    import concourse.bacc as bacc
    import concourse.mybir as mybir
    import concourse.tile as tile

    f32 = mybir.dt.float32
    bf16 = mybir.dt.bfloat16
    nc = bacc.Bacc("TRN2", target_bir_lowering=False, debug=False)
    # inputs split into per-ring chunks: the first matmul needs only w cols
    # 0..511 and rest cols 0..127, so those are tiny lead chunks whose DMA
    # completion (wire + ~2 us write receipt) gates the pipeline start
    rest_d = [nc.dram_tensor(f"rest{i}", [KP, w], bf16, kind="ExternalInput")
              for i, w in enumerate(RESTW)]
    w_d = [nc.dram_tensor(f"w{i}", [KP, w], bf16, kind="ExternalInput")
           for i, w in enumerate(WSPLIT)]
    out = nc.dram_tensor("out", [ROWS, SHARD], bf16, kind="ExternalOutput")

    with tile.TileContext(nc) as tc:
        with (
            tc.tile_pool(name="inp", bufs=1) as inp,
            tc.tile_pool(name="stage", bufs=3) as stagep,
            tc.tile_pool(name="psum", bufs=4, space="PSUM") as psump,
        ):
            # balanced across the three rings so every chunk's receipt
            # (~2 us after its last byte) fires before its first consuming
            # matmul: each ring leads with the earliest-needed chunk, and
            # chunk widths are sized so consumption never outruns arrival
            # (the pipeline start is receipt-bound, not wire-bound).
            #   sync:   w0 (gates matmuls c=0-1), w2, w5
            #   scalar: rest0 (block-0 cols, gates first LDWEIGHTS), w3
            #   gpsimd: w1 (gates matmuls c=2-3), rest1, w4
            rest_sb = [inp.tile([KP, w], bf16, tag=f"rest{i}",
                                name=f"rest{i}")
                       for i, w in enumerate(RESTW)]
            w_sb = [inp.tile([KP, w], bf16, tag=f"w{i}", name=f"w{i}")
                    for i, w in enumerate(WSPLIT)]
            nc.sync.dma_start(w_sb[0][:], w_d[0][:])
            nc.scalar.dma_start(rest_sb[0][:], rest_d[0][:])
            nc.gpsimd.dma_start(w_sb[1][:], w_d[1][:])
            nc.sync.dma_start(w_sb[2][:], w_d[2][:])
            nc.scalar.dma_start(w_sb[3][:], w_d[3][:])
            nc.gpsimd.dma_start(rest_sb[1][:], rest_d[1][:])
            nc.gpsimd.dma_start(w_sb[4][:], w_d[4][:])
            nc.sync.dma_start(w_sb[5][:], w_d[5][:])

            copy_engines = [nc.vector.tensor_copy, nc.scalar.copy]
            ncopy = 0
            # pieced blocks stream column pieces from their own small
            # tiles (a piece DMA waits on its whole tile, so shared tiles
            # would gate on all 15 copies). Ramp blocks 0-2 (fine 3-chunk
            # pieces) ride the otherwise-idle gpsimd SWDGE ring so their
            # 15 descriptor pushes don't head-of-line block the sync ring;
            # block 3 (5-chunk pieces) bridges into the steady phase and
            # block 7's pieces shorten the non-overlappable tail DMA.
            # Steady blocks 4-6 use whole-row contiguous 1.92 MB DMAs
            # from a triple-buffered full-width stage.
            for tb in range(NT):
                outp = LASTP if tb == NT - 1 else TBLK
                if tb < 3:
                    pieces, ring = [3, 3, 3, 3, 3], nc.gpsimd
                elif tb == 3:
                    pieces, ring = [5, 5, 5], nc.sync
                elif tb == NT - 1:
                    # short last piece: its wire time is the only
                    # non-overlappable part of the whole out stream
                    pieces, ring = [6, 6, 3], nc.sync
                else:
                    pieces, ring = None, nc.sync
                    stage = stagep.tile([TBLK, OUTP], bf16, tag="stage",
                                        name=f"stage{tb}", bufs=3)
                    base = 0
                if tb == 0:
                    lhsT = rest_sb[0][:]
                else:
                    lhsT = rest_sb[1][:, (tb - 1) * TBLK:tb * TBLK]
                bounds = []
                if pieces is not None:
                    acc = 0
                    for p in pieces:
                        bounds.append((acc, acc + p))
                        acc += p
                else:
                    bounds = [(0, NCH)]
                for c in range(NCH):
                    if any(c == b[0] for b in bounds):
                        pi = [i for i, b in enumerate(bounds)
                              if b[0] == c][0]
                        base = c * MMN
                        if pieces is not None:
                            pw = pieces[pi] * MMN
                            stage = stagep.tile([TBLK, pw], bf16,
                                                tag=f"p{tb}_{pi}",
                                                name=f"p{tb}_{pi}",
                                                bufs=1)
                    # chunks pair up in one 2-bank psum tile so the
                    # PSUM->SBUF drain runs at FD=1024, amortizing the
                    # ~120-170 cycle per-op overhead (~10% less engine
                    # time than FD=512 drains)
                    j = c - base // MMN
                    if j % 2 == 0:
                        ps = psump.tile([TBLK, 2 * MMN], f32, tag="ps")
                    half = (j % 2) * MMN
                    wi, off = WMAP[c]
                    nc.tensor.matmul(
                        ps[:, half:half + MMN],
                        lhsT,
                        w_sb[wi][:, off:off + MMN],
                        start=True,
                        stop=True,
                    )
                    fd = min(MMN, SHARD - c * MMN)
                    pend = any(c == b[1] - 1 for b in bounds)
                    if j % 2 == 1 or pend:
                        cl = (j // 2) * 2 * MMN
                        cw = half + fd
                        copy_engines[ncopy % 2](
                            stage[:, cl:cl + cw], ps[:, :cw]
                        )
                        ncopy += 1
                    if pend and pieces is not None:
                        lo = base
                        hi = min(c * MMN + MMN, SHARD)
                        ring.dma_start(
                            out[tb * TBLK:tb * TBLK + outp, lo:hi],
                            stage[:outp, :hi - lo],
                        )
                if pieces is None:
                    nc.sync.dma_start(
                        out[tb * TBLK:tb * TBLK + outp, :],
                        stage[:outp, :SHARD],
                    )

    nc.compile()
    return nc


def _densify(v1_weights, v1_rows, v1_cols, lm_weights, lm_rows, lm_cols):
    rows = np.concatenate([
        np.asarray(v1_rows).astype(np.int64),
        np.asarray(lm_rows).astype(np.int64) + NV1,
    ])
    cols = np.concatenate([
        np.asarray(v1_cols).astype(np.int64),
        np.asarray(lm_cols).astype(np.int64),
    ])
    w = np.concatenate([
        np.asarray(v1_weights, dtype=np.float32),
        np.asarray(lm_weights, dtype=np.float32),
    ])
    W = np.bincount(rows * NBKG + cols, weights=w, minlength=NPOST * NBKG)
    return W.astype(np.float32).reshape(NPOST, NBKG)


def kernel(rest, v1_weights, v1_rows, v1_cols, lm_weights, lm_rows, lm_cols):
    import ml_dtypes

    from concourse.bass_utils import run_bass_kernel_spmd

    bf16 = ml_dtypes.bfloat16

    global _compiled
    if _compiled is None:
        _compiled = _build_module()

    W = _densify(v1_weights, v1_rows, v1_cols, lm_weights, lm_rows, lm_cols)
    w_b = W.astype(bf16)

    rest32 = np.asarray(rest, np.float32)
    rest_b = rest32.astype(bf16)

    restT = np.zeros((KP, ROWS), bf16)
    restT[:NBKG, :B * T] = rest_b.T
    rest_chunks = {}
    lo = 0
    for i, w in enumerate(RESTW):
        rest_chunks[f"rest{i}"] = np.ascontiguousarray(restT[:, lo:lo + w])
        lo += w

    in_maps = []
    for c in range(NCORES):
        plane = np.zeros((KP, OUTP), bf16)
        plane[:NBKG, :SHARD] = w_b[c * SHARD:(c + 1) * SHARD].T
        m = dict(rest_chunks)
        lo = 0
        for i, w in enumerate(WSPLIT):
            m[f"w{i}"] = np.ascontiguousarray(plane[:, lo:lo + w])
            lo += w
        in_maps.append(m)

    trace = bool(int(os.environ.get("KERNEL_TRACE", "0")))
    if trace:
        _install_ntff_shim()
    res = run_bass_kernel_spmd(
        _compiled, in_maps, core_ids=list(range(NCORES)), trace=trace
    )
    kernel.last_results = res
    full = np.concatenate(
        [res.results[c]["out"][:B * T, :].astype(np.float32)
         for c in range(NCORES)], axis=1
    )

    # sparse host correction for any rest value that bf16 can't represent
    # exactly (Poisson counts are small ints, so normally there are none)
    rest_err = rest32 - rest_b.astype(np.float32)
    if np.any(rest_err):
        ts, ks = np.nonzero(rest_err)
        for t, k in zip(ts, ks):
            full[t, :] += rest_err[t, k] * W[:, k]

    return full.reshape(B, T, NPOST)


def _install_ntff_shim():
    """The agent image's antenv lacks axon_hooks; register the NTFF profile
    hook by dlopening libaxon_pjrt.so directly (same path trn_boot uses)."""
    import sys
    import types

    if "antenv.axon_hooks" in sys.modules:
        return
    try:
        from trn_agent_boot.trn_boot import _ntff_profile_via_ctypes

        hook = _ntff_profile_via_ctypes("/opt/axon/libaxon_pjrt.so")
    except Exception:
        hook = None
    mod = types.ModuleType("antenv.axon_hooks")
    mod.get_axon_ntff_profile_hook = lambda: hook
    mod.set_axon_ntff_profile_hook = lambda h: None
    sys.modules["antenv.axon_hooks"] = mod
